# revision 1
# baseline (speedup 1.0000x reference)
"""Trainium2 Bass kernel for ConvMultiHeadAttention (N=16, L=1024, E=512, H=8).

Data-parallel over batch: 8 NeuronCores x 2 batches each. Per core:
transposed-layout projections, S^T = K_h^T-contract-d Q_h attention logits,
softmax-over-partitions via an appended ones column in the AV matmul,
reciprocal via custom fast-DVE op (keeps ScalarE exp-table resident),
selector-matmul partition broadcast, fused output projection + bias.

Perf structure vs v1:
  - S matmuls for head pairs (2h, 2h+1) are emitted adjacently at PE row
    groups 0-63 / 64-127 so they stream concurrently (K=64 row tiling).
  - prep(b1) (DMA/cast/transpose/projections) is interleaved into the
    exp-bound attention(b0) emission so the PE never idles; outproj(b0)
    interleaves into attention(b1).
  - ScalarE runs only Exp (no Ln -> no activation-table thrash);
    reciprocal on DVE (reciprocal_approx_fast) + fp16 cast.
  - prep(b0) copies on ScalarE (idle then); prep(b1) copies on DVE
    (ScalarE is exp-bound during attention).
"""

import numpy as np
import concourse.bass as bass
import concourse.mybir as mybir
import concourse.tile as tile
from contextlib import ExitStack
from concourse import bacc

P = 128
L = 1024
E = 512
H = 8
D = 64
NB = 2            # batches per core
TT = L // P       # 8 token tiles per batch
EPO = E // P      # 4 e-subtiles
FP32 = mybir.dt.float32
FP16 = mybir.dt.float16
AF = mybir.ActivationFunctionType
ALU = mybir.AluOpType


def host_constants():
    ident16 = np.eye(P, dtype=np.float16)
    ident32 = np.eye(P, dtype=np.float32)
    # sel2[32*(h%4), 64h + j] = 1: picks denom row of head h
    sel2 = np.zeros((P, H * D), np.float16)
    for h in range(H):
        sel2[32 * (h % 4), h * D:(h + 1) * D] = 1.0
    return ident16, ident32, sel2


def build(debug=False):
    nc = bacc.Bacc("TRN2", target_bir_lowering=False, debug=debug)
    q_d = nc.dram_tensor("q", [NB, L, E], FP32, kind="ExternalInput").ap()
    k_d = nc.dram_tensor("k", [NB, L, E], FP32, kind="ExternalInput").ap()
    v_d = nc.dram_tensor("v", [NB, L, E], FP32, kind="ExternalInput").ap()
    wq_d = nc.dram_tensor("Wq", [E, E], FP32, kind="ExternalInput").ap()
    wk_d = nc.dram_tensor("Wk", [E, E], FP32, kind="ExternalInput").ap()
    wv_d = nc.dram_tensor("Wv", [E, E], FP32, kind="ExternalInput").ap()
    wo_d = nc.dram_tensor("Wo", [E, E], FP32, kind="ExternalInput").ap()
    bo_d = nc.dram_tensor("bo_bcast", [P, E], FP32, kind="ExternalInput").ap()
    id16_d = nc.dram_tensor("ident16", [P, P], FP16, kind="ExternalInput").ap()
    id32_d = nc.dram_tensor("ident32", [P, P], FP32, kind="ExternalInput").ap()
    sel_d = nc.dram_tensor("sel2", [P, H * D], FP16, kind="ExternalInput").ap()
    out_d = nc.dram_tensor("out", [NB, L, E], FP32, kind="ExternalOutput").ap()
    x_d = {"q": q_d, "k": k_d, "v": v_d}

    with tile.TileContext(nc) as tc, ExitStack() as ctx:
        consts = ctx.enter_context(tc.tile_pool(name="consts", bufs=1))
        wt_pool = ctx.enter_context(tc.tile_pool(name="wt", bufs=1))
        xin_pool = ctx.enter_context(tc.tile_pool(name="xin", bufs=4))
        xt_pool = ctx.enter_context(tc.tile_pool(name="xt", bufs=3))
        qk_pool = ctx.enter_context(tc.tile_pool(name="qk", bufs=2))
        vh_pool = ctx.enter_context(tc.tile_pool(name="vh", bufs=2))
        st_pool = ctx.enter_context(tc.tile_pool(name="st", bufs=2))
        p_pool = ctx.enter_context(tc.tile_pool(name="pp", bufs=24))
        dn_pool = ctx.enter_context(tc.tile_pool(name="dn", bufs=2))
        o_pool = ctx.enter_context(tc.tile_pool(name="oo", bufs=2))
        ps_mm = ctx.enter_context(tc.tile_pool(name="psmm", bufs=2, space="PSUM"))
        ps_s = ctx.enter_context(tc.tile_pool(name="pss", bufs=2, space="PSUM"))
        ps_o = ctx.enter_context(tc.tile_pool(name="pso", bufs=2, space="PSUM"))

        # ---- constants ----
        ident = consts.tile([P, P], FP16)
        nc.sync.dma_start(ident[:], id16_d)
        ident32 = consts.tile([P, P], FP32)
        nc.sync.dma_start(ident32[:], id32_d)
        sel = consts.tile([P, H * D], FP16)
        nc.sync.dma_start(sel[:], sel_d)
        bo_t = consts.tile([P, E], FP32)
        nc.sync.dma_start(bo_t[:], bo_d)

        early_units = []  # first k/q x-tiles: DMA+transpose before weights
        wts = {}

        def emit_weights():
            # ---- weight transposes: W [f, e] -> WT [e(pi), epo, f] fp16 ----
            # fp32 transpose (2 cyc/row) avoids a separate bf16 pre-cast pass.
            for wname, w_d in [("q", wq_d), ("k", wk_d), ("v", wv_d), ("o", wo_d)]:
                w_raw = xt_pool.tile([P, EPO, E], FP32, tag="xt",
                                     name=f"wraw_{wname}")
                nc.sync.dma_start(w_raw[:], w_d.rearrange("(fo fi) e -> fi fo e", fi=P))
                wt = wt_pool.tile([P, EPO, E], FP16, tag=f"wt_{wname}",
                                  name=f"wt_{wname}")
                for epo in range(EPO):
                    ps = ps_mm.tile([P, E], FP32, tag="mm", name=f"wps_{wname}_{epo}")
                    for fpo in range(EPO):
                        nc.tensor.transpose(
                            ps[:, fpo * P:(fpo + 1) * P],
                            w_raw[:, fpo, epo * P:(epo + 1) * P],
                            ident32[:],
                        )
                    if wname == "q":
                        nc.vector.tensor_scalar_mul(wt[:, epo, :], ps[:], 1.0 / np.sqrt(D))
                    else:
                        nc.vector.tensor_copy(wt[:, epo, :], ps[:])
                wts[wname] = wt

        # per-batch state
        xts = [dict() for _ in range(NB)]
        qkts = [dict() for _ in range(NB)]
        vhs = [None] * NB
        stages = [None] * NB
        denoms = [None] * NB
        recips = [None] * NB

        def ensure_xt(b, tname):
            if tname not in xts[b]:
                xts[b][tname] = xt_pool.tile([P, EPO, L], FP16, tag="xt", name=f"xt_{b}_{tname}")
            return xts[b][tname]

        alt_state = [0]

        def pick_act(on_act):
            if on_act == "alt":
                alt_state[0] ^= 1
                return bool(alt_state[0])
            return on_act

        def emit_xtile(b, tname, tt, on_act):
            """DMA + cast + transpose of one [128, E] tile of input tensor."""
            on_act = pick_act(on_act)
            xt = ensure_xt(b, tname)
            xin = xin_pool.tile([P, E], FP32, tag="xin")
            nc.sync.dma_start(xin[:], x_d[tname][b, tt * P:(tt + 1) * P, :])
            xin_b = xin_pool.tile([P, E], FP16, tag="xinb")
            nc.vector.tensor_copy(xin_b[:], xin[:])
            ps = ps_mm.tile([P, E], FP16, tag="mm")
            for epo in range(EPO):
                nc.tensor.transpose(
                    ps[:, epo * P:(epo + 1) * P],
                    xin_b[:, epo * P:(epo + 1) * P],
                    ident[:],
                )
            dst = xt[:, :, tt * P:(tt + 1) * P]
            src = ps[:].rearrange("p (epo t) -> p epo t", epo=EPO)
            if on_act:
                nc.scalar.copy(dst, src)
            else:
                nc.vector.tensor_copy(dst, src)

        def emit_qk_proj(b, tname, fpo, tch, on_act):
            """One [128, 512] chunk of qh^T / kh^T: [f(pi), fpo, tch]."""
            on_act = pick_act(on_act)
            wt = wts[tname]
            xt = xts[b][tname]
            if tname not in qkts[b]:
                qkts[b][tname] = qk_pool.tile([P, EPO, L], FP16, tag=f"ht_{tname}", name=f"ht_{b}_{tname}")
            ht = qkts[b][tname]
            ps = ps_mm.tile([P, E], FP32, tag="mm")
            for epo in range(EPO):
                nc.tensor.matmul(
                    ps[:],
                    wt[:, epo, fpo * P:(fpo + 1) * P],
                    xt[:, epo, tch * E:(tch + 1) * E],
                    start=(epo == 0),
                    stop=(epo == EPO - 1),
                )
            dst = ht[:, fpo, tch * E:(tch + 1) * E]
            if on_act:
                nc.scalar.copy(dst, ps[:])
            else:
                nc.vector.tensor_copy(dst, ps[:])

        def emit_vh_init(b):
            vh = vh_pool.tile([P, TT, H, D + 1], FP16, tag="vh")
            vhs[b] = vh
            # ones column at [:, :, :, D] (strided memset; rest overwritten)
            nc.vector.memset(vh[:, :, :, D:D + 1], 1.0)

        def emit_vh(b, tt, on_act):
            """vh natural [t(pi), tt, h, 65]; col 64 = ones."""
            vh = vhs[b]
            wt = wts["v"]
            xt = xts[b]["v"]
            ps = ps_mm.tile([P, E], FP32, tag="mm")
            for epo in range(EPO):
                nc.tensor.matmul(
                    ps[:],
                    xt[:, epo, tt * P:(tt + 1) * P],
                    wt[:, epo, :],
                    start=(epo == 0),
                    stop=(epo == EPO - 1),
                )
            dst = vh[:, tt, :, 0:D]
            src = ps[:].rearrange("p (h d) -> p h d", h=H)
            if on_act:
                nc.scalar.copy(dst, src)
            else:
                nc.vector.tensor_copy(dst, src)

        # ---------- work queue of deferred prep units ----------
        # entries: (label_or_None, fn); label marks group completion AFTER fn
        queue = []
        done_labels = set()

        def _run(entry):
            label, fn = entry
            fn()
            if label is not None:
                done_labels.add(label)

        def pump(n):
            for _ in range(min(n, len(queue))):
                _run(queue.pop(0))

        def drain_until(label):
            while label not in done_labels and queue:
                _run(queue.pop(0))

        # ---------- attention ----------
        def emit_s_exp_pair(b, hpo, lt):
            """S^T logits for heads (2*hpo, 2*hpo+1) on k-tile lt, then exp.
            The two heads sit at PE row groups 0-63 / 64-127 -> concurrent."""
            qht, kht = qkts[b]["q"], qkts[b]["k"]
            pss = [ps_s.tile([P, L], FP32, tag="s", name=f"pss_{b}_{hpo}_{lt}_{i}") for i in range(2)]
            for ch in range(L // E):
                for hh in range(2):
                    hoff = D * hh
                    nc.tensor.matmul(
                        pss[hh][:, ch * E:(ch + 1) * E],
                        kht[hoff:hoff + D, hpo, lt * P:(lt + 1) * P],
                        qht[hoff:hoff + D, hpo, ch * E:(ch + 1) * E],
                        start=True,
                        stop=True,
                    )
            pts = []
            for hh in range(2):
                pt = p_pool.tile([P, L], FP16, tag="p")
                nc.scalar.activation(pt[:], pss[hh][:], AF.Exp)
                pts.append(pt)
            return pts

        def emit_av_copies(b, h, pso, ch):
            """stage + denom copies out of an AV psum tile for chunk ch."""
            stage = stages[b]
            denom = denoms[b]
            hpo, hoff = h // 2, D * (h % 2)
            nc.vector.tensor_copy(
                stage[hoff:hoff + D, hpo, ch * E:(ch + 1) * E], pso[0:D, :]
            )
            nc.vector.tensor_copy(
                denom[32 * (h % 4):32 * (h % 4) + 1, h // 4, ch * E:(ch + 1) * E],
                pso[D:D + 1, :],
            )

        def emit_av_ch(b, h, pts_lt, ch, interleave=()):
            """O^T accumulation for head h, chunk ch; psum row 64 = denom."""
            vh = vhs[b]
            inter = list(interleave)
            pso = ps_o.tile([D + 1, E], FP32, tag="o")
            for lt in range(TT):
                nc.tensor.matmul(
                    pso[:],
                    vh[:, lt, h, :],
                    pts_lt[lt][:, ch * E:(ch + 1) * E],
                    start=(lt == 0),
                    stop=(lt == TT - 1),
                )
            emit_av_copies(b, h, pso, ch)
            while inter:
                inter.pop(0)()

        def emit_recip(b, half):
            """recip = 1/denom on DVE (ScalarE keeps its Exp table loaded).
            half 0 covers heads 0-3 (ready after pair 1), half 1 heads 4-7."""
            denom = denoms[b]
            if recips[b] is None:
                recips[b] = dn_pool.tile([P, 2, L], FP16, tag="dnr", bufs=2,
                                         name=f"recip_{b}")
            recip = recips[b]
            r32 = dn_pool.tile([P, L], FP32, tag="dnr32", bufs=1,
                               name=f"r32_{b}_{half}")
            nc.vector.reciprocal_approx_fast(r32[:], denom[:, half, :])
            nc.vector.tensor_copy(recip[:, half, :], r32[:])

        def emit_norm_head(b, h):
            """stage[head h] *= broadcast(recip[h]) via selector matmul."""
            stage = stages[b]
            recip = recips[b]
            hpo, hoff = h // 2, D * (h % 2)
            for ch in range(L // E):
                psb = ps_o.tile([D, E], FP32, tag="o")
                nc.tensor.matmul(
                    psb[:],
                    sel[:, h * D:(h + 1) * D],
                    recip[:, h // 4, ch * E:(ch + 1) * E],
                    start=True,
                    stop=True,
                )
                nc.vector.tensor_tensor(
                    stage[hoff:hoff + D, hpo, ch * E:(ch + 1) * E],
                    psb[:],
                    stage[hoff:hoff + D, hpo, ch * E:(ch + 1) * E],
                    ALU.mult,
                )

        def emit_outproj(b, tt):
            stage = stages[b]
            wt = wts["o"]
            ps = ps_mm.tile([P, E], FP32, tag="mm")
            for epo in range(EPO):
                nc.tensor.matmul(
                    ps[:],
                    stage[:, epo, tt * P:(tt + 1) * P],
                    wt[:, epo, :],
                    start=(epo == 0),
                    stop=(epo == EPO - 1),
                )
            ot = o_pool.tile([P, E], FP32, tag="ot")
            nc.vector.tensor_tensor(ot[:], ps[:], bo_t[:], ALU.add)
            # alternate DMA queues so the final output tiles drain in parallel
            eng = nc.gpsimd if tt % 2 == 0 else nc.sync
            eng.dma_start(out_d[b, tt * P:(tt + 1) * P, :], ot[:])

        def prep_phase1_units(b, on_act):
            u = []
            for tname in ("k", "q"):
                for tt in range(TT):
                    u.append((None, lambda b=b, t=tname, tt=tt: emit_xtile(b, t, tt, on_act)))
            for i, (tname, tch) in enumerate(
                    [(t, c) for t in ("k", "q") for c in range(L // E)]):
                lbl = ("proj", b, 0) if i == 3 else None
                u.append((lbl, lambda b=b, t=tname, tch=tch: emit_qk_proj(b, t, 0, tch, on_act)))
            return u

        def prep_phase2_units(b, on_act):
            u = [(None, lambda b=b: emit_vh_init(b))]
            for tt in range(TT):
                u.append((None, lambda b=b, tt=tt: emit_xtile(b, "v", tt, on_act)))
            for tt in range(TT):
                lbl = ("vh", b) if tt == TT - 1 else None
                u.append((lbl, lambda b=b, tt=tt: emit_vh(b, tt, on_act)))
            return u

        def proj_units(b, fpo, on_act):
            u = []
            for i, (tname, tch) in enumerate(
                    [(t, c) for t in ("k", "q") for c in range(L // E)]):
                lbl = ("proj", b, fpo) if i == 3 else None
                u.append((lbl, lambda b=b, t=tname, f=fpo, tch=tch: emit_qk_proj(b, t, f, tch, on_act)))
            return u

        def attn_pair(b, hpo, tail_units):
            """S/exp/AV for heads (2hpo, 2hpo+1) of batch b, pumping queue
            (then tail_units) into PE bubbles."""
            if stages[b] is None:
                stages[b] = st_pool.tile([P, EPO, L], FP16, tag="st", name=f"stage_{b}")
                denoms[b] = dn_pool.tile([P, 2, L], FP32, tag="dn", name=f"denom_{b}", bufs=2)
                nc.vector.memset(denoms[b][:], 1.0)
            drain_until(("proj", b, hpo))
            drain_until(("vh", b))
            vh = vhs[b]
            pso0 = [ps_o.tile([D + 1, E], FP32, tag="o", name=f"pso0_{b}_{hpo}_{i}")
                    for i in range(2)]
            pts_pair = [[], []]
            for lt in range(TT):
                pts = emit_s_exp_pair(b, hpo, lt)
                for hh in range(2):
                    pts_pair[hh].append(pts[hh])
                    nc.tensor.matmul(
                        pso0[hh][:],
                        vh[:, lt, 2 * hpo + hh, :],
                        pts[hh][:, 0:E],
                        start=(lt == 0),
                        stop=(lt == TT - 1),
                    )
                pump(4)
            for hh in range(2):
                emit_av_copies(b, 2 * hpo + hh, pso0[hh], 0)
            # AV ch1; interleave queued units (or tails) into its slots
            for hh in range(2):
                h = 2 * hpo + hh
                inter = []
                for _ in range(2):
                    if queue:
                        inter.append(lambda e=queue.pop(0): _run(e))
                    elif tail_units:
                        inter.append(tail_units.pop(0))
                emit_av_ch(b, h, pts_pair[hh], 1, interleave=inter)

        # =================== schedule ===================
        # prep(b0) phase1 inline (fill); copies alternate ACT/DVE so neither
        # queue backlog gates the first S matmuls
        emit_weights()
        for u in prep_phase1_units(0, on_act="alt"):
            _run(u)
        # b0's remaining projections inline: dense PE work -> HAM warms up
        # before attention instead of flapping through the DMA-bound prep
        for fpo in range(1, EPO):
            for u in proj_units(0, fpo, on_act="alt"):
                _run(u)
        queue.extend(prep_phase2_units(0, on_act=True))
        queue.extend(prep_phase1_units(1, on_act=False))
        queue.extend(prep_phase2_units(1, on_act=False))
        for fpo in range(1, EPO):
            queue.extend(proj_units(1, fpo, on_act=False))

        tails = []
        for b in range(NB):
            for hpo in range(H // 2):
                attn_pair(b, hpo, tails)
                if hpo == 1:
                    # heads 0-3 done: their denom half + norms can go
                    tails += [lambda b=b: emit_recip(b, 0)]
                    tails += [lambda b=b, h=h: emit_norm_head(b, h)
                              for h in range(4)]
                elif hpo == H // 2 - 1:
                    tails += [lambda b=b: emit_recip(b, 1)]
                    tails += [lambda b=b, h=h: emit_norm_head(b, h)
                              for h in range(4, H)]
                    tails += [lambda b=b, tt=tt: emit_outproj(b, tt)
                              for tt in range(TT)]
        for u in tails:
            u()

    nc.compile()
    return nc


_COMPILED = None


def _get_compiled():
    global _COMPILED
    if _COMPILED is None:
        _COMPILED = build()
    return _COMPILED


def kernel(q, k, v, Wq, Wk, Wv, Wo, bo):
    import numpy as _np

    q = _np.ascontiguousarray(_np.asarray(q, dtype=_np.float32))
    k = _np.ascontiguousarray(_np.asarray(k, dtype=_np.float32))
    v = _np.ascontiguousarray(_np.asarray(v, dtype=_np.float32))
    Wq = _np.ascontiguousarray(_np.asarray(Wq, dtype=_np.float32))
    Wk = _np.ascontiguousarray(_np.asarray(Wk, dtype=_np.float32))
    Wv = _np.ascontiguousarray(_np.asarray(Wv, dtype=_np.float32))
    Wo = _np.ascontiguousarray(_np.asarray(Wo, dtype=_np.float32))
    bo = _np.asarray(bo, dtype=_np.float32)

    nc = _get_compiled()
    ident16, ident32, sel2 = host_constants()
    bo_bcast = _np.ascontiguousarray(_np.broadcast_to(bo, (P, E)))
    n_cores = 8
    in_maps = []
    for c in range(n_cores):
        in_maps.append({
            "q": _np.ascontiguousarray(q[c * NB:(c + 1) * NB]),
            "k": _np.ascontiguousarray(k[c * NB:(c + 1) * NB]),
            "v": _np.ascontiguousarray(v[c * NB:(c + 1) * NB]),
            "Wq": Wq, "Wk": Wk, "Wv": Wv, "Wo": Wo,
            "bo_bcast": bo_bcast, "ident16": ident16, "ident32": ident32,
            "sel2": sel2,
        })

    from concourse.bass_utils import run_bass_kernel_spmd
    res = run_bass_kernel_spmd(nc, in_maps, core_ids=list(range(n_cores)))
    out = _np.concatenate([res.results[c]["out"] for c in range(n_cores)], axis=0)
    return out.astype(_np.float32)

